# revision 11
# baseline (speedup 1.0000x reference)
"""Trainium2 Bass kernel for nn_DAC_structure (sparse dual-attention structure map).

For inputs q/k of shape (B*CH, L, H, E) = (64, 32, 8, 64):
  s  = softmax((q @ k^T) / sqrt(E))            per (batch-channel, head)
  m  = mean over the CH=8 channel group        -> [b, H, 32, 32]
  out_ps = element-repeat(m_ps, 32, 32)        -> [b, H, 1024, 1024]
  out_pn = tile(m_pn, 32, 32)                  -> [b, H, 1024, 1024]

Sharding: data-parallel over the true batch dim b = 8; core i handles batch i
(channel rows 8i..8i+8). No cross-device comms. Each core writes its own
[8, 1024, 1024] x2 output shard; the host stacks shards along axis 0.

The kernel is HBM-write-bound. The HBM port per NeuronCore caps at ~358 GB/s
(716 GB/s per stack, 2 NCs/stack, all 8 cores active), so the f32 output
(64 MB/core) floors at ~187 us. This version writes fp16 instead (32 MB/core,
~94 us floor; rel err ~5e-4 vs the 2e-2 gate) and upcasts on the host during
the gather. Structure:
  - All output DMAs walk HBM fully sequentially: out_ps re-reads each source
    row 32x via a stride-0 mid AP dim; out_pn re-reads a partition-replicated
    [128, 1024] tile 8x via a stride-0 OUTER AP dim (dst merges to one
    contiguous 2 MB walk per head).
  - out_pn partition replication (32 rows -> 128) is done on the PE with a
    block-replicated identity (rep = R^T @ exp), not SBUF->SBUF DMA, so the
    SDMA engines spend all their time on HBM writes.
  - QK^T matmuls run in fp16 (tr tiles cast during the PSUM->SBUF transpose
    copy); softmax + channel-mean stay f32.
  - Everything is issued so the sync-ring write stream starts as soon as the
    first h-group's ps tile is ready (~12 us in) and never gaps.
"""

import sys

if "/opt/trn_rl_repo" not in sys.path:
    sys.path.insert(0, "/opt/trn_rl_repo")

from contextlib import ExitStack

import numpy as np

import concourse.bacc as bacc
import concourse.bass as bass
import concourse.mybir as mybir
import concourse.tile as tile
from concourse.masks import make_identity

F32 = mybir.dt.float32
F16 = mybir.dt.float16

CH = 8   # channels per true batch
L = 32   # patch_num (seq len of the small attention)
H = 8    # heads
E = 64   # head dim
WIN = 1024
N_CORES = 8


def _load_inputs(nc, pool, ins, kind):
    """Two DMAs (one per channel-half) per tensor into
    [128 = (c%4)*32 + l, 1024 = (c//4)*512 + h*64 + e]; q on the Scalar ring,
    k on the Sync ring (ahead of the output writes) so the load drains 2x
    faster and the first transpose starts earlier."""
    nat = {}
    for name, eng in (("q", nc.scalar), ("k", nc.sync)):
        dram = ins[name]
        t = pool.tile([128, 1024], F32, tag=f"nat_{kind}_{name}", name=f"nat_{kind}_{name}")
        for chalf in range(2):
            src = bass.AP(tensor=dram.tensor, offset=dram.offset + chalf * 4 * L * H * E,
                          ap=[[H * E, 128], [1, H * E]])
            eng.dma_start(out=t[:, chalf * 512 : (chalf + 1) * 512], in_=src)
        nat[name] = t
    return nat


def _make_repmat(nc, pool, ident):
    """R2j [128, 128] f16 (j=0,1) with R2j[q, p] = (q%32 == (2p+j)%32): lhsT
    of the pn partition-replication matmuls. rep2 tile row content for
    partition p, half j is exp row (2p+j)%32, so a [128, 2048] tile covers
    256 consecutive output rows."""
    ipitch = ident.ap[0][0]
    rs = []
    for j in range(2):
        r = pool.tile([128, 128], F16, tag=f"repmat{j}", name=f"repmat{j}")
        rpitch = r.ap[0][0]
        for jb in range(4):
            src = bass.AP(tensor=ident.tensor,
                          offset=ident.offset + jb * 32 * ipitch + jb * 32 + j,
                          ap=[[ipitch, 32], [0, 8], [2, 16]])
            dst = bass.AP(tensor=r.tensor, offset=r.offset + jb * 32 * rpitch,
                          ap=[[rpitch, 32], [16, 8], [1, 16]])
            nc.vector.tensor_copy(dst, src)
        rs.append(r)
    return rs


def _transpose_group(nc, psum_t, ident, nat, tr, g):
    """PE-transpose the 128-col chunks needed by h-group g (h = 4g..4g+3).
    The PSUM->SBUF copy casts f32 -> f16 so the QK^T matmuls run in fp16."""
    js = (0, 1, 4, 5) if g == 0 else (2, 3, 6, 7)
    for name in ("q", "k"):
        for j in js:
            pt = psum_t.tile([128, 128], F32, tag="ptrans", name="ptrans")
            nc.tensor.transpose(pt, nat[name][:, j * 128 : (j + 1) * 128], ident)
            nc.vector.tensor_copy(tr[name][:, j * 128 : (j + 1) * 128], pt)


def _group_mean_softmax(nc, pool, psum_s, tr, g, kind):
    """QK^T matmuls + softmax + channel mean for h-group g -> M [128, 32]."""
    s_ps = psum_s.tile([128, 256], F32, tag="spsum", name="spsum")
    for c in range(CH):
        chalf, clo = divmod(c, 4)
        for hh in range(4):
            h = g * 4 + hh
            col = (chalf * 4 + h // 2) * 128 + clo * 32
            prow = (h % 2) * 64
            nc.tensor.matmul(
                s_ps[hh * 32 : hh * 32 + 32, c * 32 : c * 32 + 32],
                tr["q"][prow : prow + 64, col : col + 32],
                tr["k"][prow : prow + 64, col : col + 32],
                start=True, stop=True,
                tile_position=(prow, hh * 32),
            )
    ex = pool.tile([128, 256], F32, tag=f"ex_{kind}", name=f"ex_{kind}")
    nc.scalar.activation(ex, s_ps, mybir.ActivationFunctionType.Exp, scale=1.0 / 8.0)
    r = pool.tile([128, 8], F32, tag=f"r_{kind}", name=f"r_{kind}")
    ex_cview = bass.AP(tensor=ex.tensor, offset=ex.offset,
                       ap=[list(ex.ap[0]), [32, 8], [1, 32]])
    nc.vector.tensor_reduce(r, ex_cview, axis=mybir.AxisListType.X, op=mybir.AluOpType.add)
    w = pool.tile([128, 8], F32, tag=f"w_{kind}", name=f"w_{kind}")
    nc.vector.reciprocal(w, r)
    wx = pool.tile([128, 256], F32, tag=f"wx_{kind}", name=f"wx_{kind}")
    ex_scl = bass.AP(tensor=ex.tensor, offset=ex.offset,
                     ap=[list(ex.ap[0]), [1, 32], [32, 8]])
    w_bc = bass.AP(tensor=w.tensor, offset=w.offset,
                   ap=[list(w.ap[0]), [0, 32], [1, 8]])
    wx_out = bass.AP(tensor=wx.tensor, offset=wx.offset,
                     ap=[list(wx.ap[0]), [8, 32], [1, 8]])
    nc.vector.scalar_tensor_tensor(out=wx_out, in0=ex_scl, scalar=1.0 / CH, in1=w_bc,
                                   op0=mybir.AluOpType.mult, op1=mybir.AluOpType.mult)
    m = pool.tile([128, 32], F32, tag=f"m_{kind}_{g}", name=f"m_{kind}_{g}")
    wx_in = bass.AP(tensor=wx.tensor, offset=wx.offset,
                    ap=[list(wx.ap[0]), [8, 32], [1, 8]])
    nc.vector.tensor_reduce(m, wx_in, axis=mybir.AxisListType.X, op=mybir.AluOpType.add)
    return m


def _build_ps_tile(nc, pool, m, g):
    """Element-repeat expansion tile [128, 2048] f16 (two identical 1024-col
    copies) for h-group g: doubles the DMA descriptor size to 4 KiB."""
    exp_t = pool.tile([128, 2048], F16, tag=f"expand_ps_{g}", name=f"expand_ps_{g}")
    src = bass.AP(tensor=m.tensor, offset=m.offset,
                  ap=[list(m.ap[0]), [1, 32], [0, 32]])
    for copy in range(2):
        nc.vector.tensor_copy(exp_t[:, copy * 1024 : (copy + 1) * 1024], src)
    return exp_t


def _build_pn_tiles(nc, pool, psum_r, repmats, m, g):
    """Tile expansion [128, 1024] f16 + per-head partition replication on the
    PE: rep2[p, j*1024 + c] = exp[hh*32 + (2p+j)%32, c] via two matmuls with
    the R2j permuted identities, then DVE PSUM->SBUF f16 copies. Each rep2
    tile holds 256 consecutive output rows. No SDMA involvement."""
    exp_t = pool.tile([128, 1024], F16, tag=f"expand_pn_{g}", name=f"expand_pn_{g}")
    src = bass.AP(tensor=m.tensor, offset=m.offset,
                  ap=[list(m.ap[0]), [0, 32], [1, 32]])
    nc.vector.tensor_copy(exp_t, src)
    reps = []
    for hh in range(4):
        rep = pool.tile([128, 2048], F16, tag=f"rep_pn_{g * 4 + hh}",
                        name=f"rep_pn_{g * 4 + hh}")
        for j in range(2):
            pr = psum_r.tile([128, 1024], F32, tag="rep_psum", name="rep_psum")
            for half in range(2):
                nc.tensor.matmul(
                    pr[:, half * 512 : (half + 1) * 512],
                    repmats[j][hh * 32 : (hh + 1) * 32, :],
                    exp_t[hh * 32 : (hh + 1) * 32, half * 512 : (half + 1) * 512],
                    start=True, stop=True,
                    tile_position=(hh * 32, 0),
                )
            nc.vector.tensor_copy(rep[:, j * 1024 : (j + 1) * 1024], pr)
        reps.append(rep)
    return reps


def _write_ps_group(nc, exp_t, out_dram, g, nparts=128):
    """One DMA per h-group: each 2-row source line re-read 16x (stride-0 mid
    dim) -> 32 identical consecutive output rows per partition; the HBM walk
    is fully sequential over the group's [nparts*32, 1024] row span."""
    pitch = exp_t.ap[0][0]
    src = bass.AP(tensor=exp_t.tensor, offset=exp_t.offset,
                  ap=[[pitch, nparts], [0, 16], [1, 2048]])
    dst = bass.AP(tensor=out_dram.tensor,
                  offset=out_dram.offset + g * 4 * WIN * WIN,
                  ap=[[32 * WIN, nparts], [2 * WIN, 16], [1, 2 * WIN]])
    nc.sync.dma_start(out=dst, in_=src)


def _make_dupmat(nc, pool, ident):
    """Rd [128, 64] f16 on partitions 96-127: Rd[96+s, 16a+b] = (s == 28+a).
    lhsT of the tail-duplicate matmul: dup[q] = exp_ps_g1[124 + q//16]."""
    rd = pool.tile([128, 64], F16, tag="dupmat", name="dupmat")
    ipitch = ident.ap[0][0]
    rpitch = rd.ap[0][0]
    src = bass.AP(tensor=ident.tensor, offset=ident.offset + 96 * ipitch + 124,
                  ap=[[ipitch, 32], [1, 4], [0, 16]])
    dst = bass.AP(tensor=rd.tensor, offset=rd.offset + 96 * rpitch,
                  ap=[[rpitch, 32], [16, 4], [1, 16]])
    nc.vector.tensor_copy(dst, src)
    return rd


def _build_dup_tile(nc, pool, psum_r, dupmat, exp_t):
    """dup [64, 2048] f16 with dup[q] = exp_ps_g1[124 + q//16]: the content of
    output rows 3968..4095 of h-group 1 (head 7, rows 896..1023), stored on
    partitions 0-63 (the 8 even SDMA ports) to shed load from slow port 15."""
    dup = pool.tile([64, 2048], F16, tag="dup_ps", name="dup_ps")
    for r in range(2):
        pr = psum_r.tile([128, 1024], F32, tag="rep_psum", name="rep_psum")
        for half in range(2):
            nc.tensor.matmul(
                pr[0:64, half * 512 : (half + 1) * 512],
                dupmat[96:128, :],
                exp_t[96:128, r * 1024 + half * 512 : r * 1024 + (half + 1) * 512],
                start=True, stop=True,
                tile_position=(96, 0),
            )
        nc.vector.tensor_copy(dup[:, r * 1024 : (r + 1) * 1024], pr[0:64, :])
    return dup


def _write_ps_dup(nc, dup, out_dram):
    """0.25 MB tail DMA: rows 8064..8191 of out_ps (head 7 rows 896..1023)
    from the even-port dup tile; fully sequential dst walk."""
    pitch = dup.ap[0][0]
    src = bass.AP(tensor=dup.tensor, offset=dup.offset,
                  ap=[[pitch, 64], [1, 2048]])
    dst = bass.AP(tensor=out_dram.tensor,
                  offset=out_dram.offset + (4096 + 3968) * WIN,
                  ap=[[2 * WIN, 64], [1, 2 * WIN]])
    nc.sync.dma_start(out=dst, in_=src)


def _write_pn_head(nc, rep, out_dram, h):
    """One 2 MB DMA per head: the [128, 2048] rep2 tile (256 rows) re-read 4x
    via a stride-0 mid src dim. The dst walk is 4 interleaved sequential
    streams of 4 KB runs (partition p writes rows 2p,2p+1 of each 256-row
    band), which HBM handles near-sequentially."""
    pitch = rep.ap[0][0]
    src = bass.AP(tensor=rep.tensor, offset=rep.offset,
                  ap=[[pitch, 128], [0, 4], [1, 2048]])
    dst = bass.AP(tensor=out_dram.tensor,
                  offset=out_dram.offset + h * WIN * WIN,
                  ap=[[2 * WIN, 128], [256 * WIN, 4], [1, 2 * WIN]])
    nc.sync.dma_start(out=dst, in_=src)


def build_program():
    """Build and compile the per-core Bass program. Returns the Bacc object."""
    nc = bacc.Bacc(
        "TRN2",
        target_bir_lowering=False,
        debug=False,
        enable_asserts=False,
        num_devices=N_CORES,
    )
    ins = {}
    for name in ("qps", "qpn", "kps", "kpn"):
        ins[name] = nc.dram_tensor(name, [CH, L, H, E], F32, kind="ExternalInput").ap()
    out_ps = nc.dram_tensor("out_ps", [H, WIN, WIN], F16, kind="ExternalOutput").ap()
    out_pn = nc.dram_tensor("out_pn", [H, WIN, WIN], F16, kind="ExternalOutput").ap()

    with tile.TileContext(nc) as tc:
        with ExitStack() as ctx:
            pool = ctx.enter_context(tc.tile_pool(name="sbuf", bufs=1))
            chunk_pool = ctx.enter_context(tc.tile_pool(name="chunks", bufs=2))
            psum_t = ctx.enter_context(tc.tile_pool(name="ptrans", bufs=2, space="PSUM"))
            psum_s = ctx.enter_context(tc.tile_pool(name="spsum", bufs=2, space="PSUM"))
            psum_r = ctx.enter_context(tc.tile_pool(name="rpsum", bufs=2, space="PSUM"))
            ident = pool.tile([128, 128], F32, tag="ident")
            make_identity(nc, ident)
            repmats = _make_repmat(nc, pool, ident)
            dupmat = _make_dupmat(nc, pool, ident)

            nat_ps = _load_inputs(nc, pool, {"q": ins["qps"], "k": ins["kps"]}, "ps")
            nat_pn = _load_inputs(nc, pool, {"q": ins["qpn"], "k": ins["kpn"]}, "pn")
            tr_ps = {n: pool.tile([128, 1024], F16, tag=f"tr_ps_{n}",
                                  name=f"tr_ps_{n}") for n in ("q", "k")}
            tr_pn = {n: pool.tile([128, 1024], F16, tag=f"tr_pn_{n}",
                                  name=f"tr_pn_{n}") for n in ("q", "k")}

            # Pipelined: each write is queued the moment its source tile
            # exists; pn rep tiles are built on PE+DVE while the sync ring
            # drains earlier writes. Stream order [ps g0, pn h0-3, pn h4-7,
            # ps g1] with the final ps group written from partitions [0,124)
            # plus an even-port dup tile: SDMA port 15 runs ~13% slower than
            # the rest, so it gets ~0.5 MB less work and all 16 ports finish
            # together instead of port 15 dribbling out a 12 us serial tail.

            # group 0: ps then pn
            _transpose_group(nc, psum_t, ident, nat_ps, tr_ps, 0)
            m = _group_mean_softmax(nc, chunk_pool, psum_s, tr_ps, 0, "ps")
            exp_ps = _build_ps_tile(nc, pool, m, 0)
            _write_ps_group(nc, exp_ps, out_ps, 0)

            _transpose_group(nc, psum_t, ident, nat_pn, tr_pn, 0)
            m = _group_mean_softmax(nc, chunk_pool, psum_s, tr_pn, 0, "pn")
            reps = _build_pn_tiles(nc, pool, psum_r, repmats, m, 0)
            for hh in range(4):
                _write_pn_head(nc, reps[hh], out_pn, hh)

            # group 1: pn first, ps (with port-15 shedding) last
            _transpose_group(nc, psum_t, ident, nat_pn, tr_pn, 1)
            m = _group_mean_softmax(nc, chunk_pool, psum_s, tr_pn, 1, "pn")
            reps = _build_pn_tiles(nc, pool, psum_r, repmats, m, 1)
            for hh in range(4):
                _write_pn_head(nc, reps[hh], out_pn, 4 + hh)

            _transpose_group(nc, psum_t, ident, nat_ps, tr_ps, 1)
            m = _group_mean_softmax(nc, chunk_pool, psum_s, tr_ps, 1, "ps")
            exp_ps = _build_ps_tile(nc, pool, m, 1)
            dup = _build_dup_tile(nc, pool, psum_r, dupmat, exp_ps)
            _write_ps_dup(nc, dup, out_ps)
            _write_ps_group(nc, exp_ps, out_ps, 1, nparts=124)
    nc.compile()
    return nc


_NC_CACHE = None


def _get_nc():
    global _NC_CACHE
    if _NC_CACHE is None:
        _NC_CACHE = build_program()
    return _NC_CACHE


def run_sharded(queries_patch_size, queries_patch_num, keys_patch_size, keys_patch_num,
                trace=False, tmpdir=None):
    """Run the SPMD kernel on 8 cores; returns (full_ps, full_pn[, results])."""
    from concourse.bass_utils import run_bass_kernel_spmd

    nc = _get_nc()
    qps = np.ascontiguousarray(np.asarray(queries_patch_size, dtype=np.float32))
    qpn = np.ascontiguousarray(np.asarray(queries_patch_num, dtype=np.float32))
    kps = np.ascontiguousarray(np.asarray(keys_patch_size, dtype=np.float32))
    kpn = np.ascontiguousarray(np.asarray(keys_patch_num, dtype=np.float32))

    in_maps = []
    for b in range(N_CORES):
        sl = slice(b * CH, (b + 1) * CH)
        in_maps.append({
            "qps": qps[sl], "qpn": qpn[sl], "kps": kps[sl], "kpn": kpn[sl],
        })
    res = run_bass_kernel_spmd(nc, in_maps, core_ids=list(range(N_CORES)), trace=trace,
                               tmpdir=tmpdir)
    full_ps = np.stack([np.asarray(res.results[b]["out_ps"]).astype(np.float32)
                        for b in range(N_CORES)], axis=0)
    full_pn = np.stack([np.asarray(res.results[b]["out_pn"]).astype(np.float32)
                        for b in range(N_CORES)], axis=0)
    if trace:
        return full_ps, full_pn, res
    return full_ps, full_pn


def kernel(queries_patch_size, queries_patch_num, keys_patch_size, keys_patch_num,
           values=None, patch_index=0, attn_mask=None):
    """Full-input entry point: takes the unsharded inputs, returns full outputs."""
    full_ps, full_pn = run_sharded(
        queries_patch_size, queries_patch_num, keys_patch_size, keys_patch_num
    )
    return full_ps, full_pn


# revision 14
# speedup vs baseline: 1.6929x; 1.6929x over previous
"""Trainium2 Bass kernel for nn_DAC_structure (sparse dual-attention structure map).

For inputs q/k of shape (B*CH, L, H, E) = (64, 32, 8, 64):
  s  = softmax((q @ k^T) / sqrt(E))            per (batch-channel, head)
  m  = mean over the CH=8 channel group        -> [b, H, 32, 32]
  out_ps = element-repeat(m_ps, 32, 32)        -> [b, H, 1024, 1024]
  out_pn = tile(m_pn, 32, 32)                  -> [b, H, 1024, 1024]

Sharding: data-parallel over the true batch dim b = 8; core i handles batch i
(channel rows 8i..8i+8). No cross-device comms. Each core writes its own
[8, 1024, 1024] x2 output shard; the host stacks shards along axis 0.

The kernel is HBM-write-bound. The HBM port per NeuronCore caps at ~358 GB/s
(716 GB/s per stack, 2 NCs/stack, all 8 cores active), so the f32 output
(64 MB/core) floors at ~187 us. This version writes fp16 instead (32 MB/core,
~94 us floor; rel err ~5e-4 vs the 2e-2 gate) and upcasts on the host during
the gather. Structure:
  - All output DMAs walk HBM fully sequentially: out_ps re-reads each source
    row 32x via a stride-0 mid AP dim; out_pn re-reads a partition-replicated
    [128, 1024] tile 8x via a stride-0 OUTER AP dim (dst merges to one
    contiguous 2 MB walk per head).
  - out_pn partition replication (32 rows -> 128) is done on the PE with a
    block-replicated identity (rep = R^T @ exp), not SBUF->SBUF DMA, so the
    SDMA engines spend all their time on HBM writes.
  - QK^T matmuls run in fp16 (tr tiles cast during the PSUM->SBUF transpose
    copy); softmax + channel-mean stay f32.
  - Everything is issued so the sync-ring write stream starts as soon as the
    first h-group's ps tile is ready (~12 us in) and never gaps.
"""

import sys

if "/opt/trn_rl_repo" not in sys.path:
    sys.path.insert(0, "/opt/trn_rl_repo")

from contextlib import ExitStack

import numpy as np

import concourse.bacc as bacc
import concourse.bass as bass
import concourse.mybir as mybir
import concourse.tile as tile
from concourse.masks import make_identity

F32 = mybir.dt.float32
F16 = mybir.dt.float16

CH = 8   # channels per true batch
L = 32   # patch_num (seq len of the small attention)
H = 8    # heads
E = 64   # head dim
WIN = 1024
N_CORES = 8


def _load_inputs(nc, pool, ins, kind):
    """Two DMAs (one per channel-half) per tensor into
    [128 = (c%4)*32 + l, 1024 = (c//4)*512 + h*64 + e]; q on the Scalar ring,
    k on the Sync ring (ahead of the output writes) so the load drains 2x
    faster and the first transpose starts earlier."""
    nat = {}
    for name, eng in (("q", nc.scalar), ("k", nc.sync)):
        dram = ins[name]
        t = pool.tile([128, 1024], F32, tag=f"nat_{kind}_{name}", name=f"nat_{kind}_{name}")
        for chalf in range(2):
            src = bass.AP(tensor=dram.tensor, offset=dram.offset + chalf * 4 * L * H * E,
                          ap=[[H * E, 128], [1, H * E]])
            eng.dma_start(out=t[:, chalf * 512 : (chalf + 1) * 512], in_=src)
        nat[name] = t
    return nat


def _make_repmat(nc, pool, ident):
    """R2j [128, 128] f16 (j=0,1) with R2j[q, p] = (q%32 == (2p+j)%32): lhsT
    of the pn partition-replication matmuls. rep2 tile row content for
    partition p, half j is exp row (2p+j)%32, so a [128, 2048] tile covers
    256 consecutive output rows."""
    ipitch = ident.ap[0][0]
    rs = []
    for j in range(2):
        r = pool.tile([128, 128], F16, tag=f"repmat{j}", name=f"repmat{j}")
        rpitch = r.ap[0][0]
        for jb in range(4):
            src = bass.AP(tensor=ident.tensor,
                          offset=ident.offset + jb * 32 * ipitch + jb * 32 + j,
                          ap=[[ipitch, 32], [0, 8], [2, 16]])
            dst = bass.AP(tensor=r.tensor, offset=r.offset + jb * 32 * rpitch,
                          ap=[[rpitch, 32], [16, 8], [1, 16]])
            nc.vector.tensor_copy(dst, src)
        rs.append(r)
    return rs


def _transpose_group(nc, psum_t, ident, nat, tr, g):
    """PE-transpose the 128-col chunks needed by h-group g (h = 4g..4g+3).
    The PSUM->SBUF copy casts f32 -> f16 so the QK^T matmuls run in fp16."""
    js = (0, 1, 4, 5) if g == 0 else (2, 3, 6, 7)
    for name in ("q", "k"):
        for j in js:
            pt = psum_t.tile([128, 128], F32, tag="ptrans", name="ptrans")
            nc.tensor.transpose(pt, nat[name][:, j * 128 : (j + 1) * 128], ident)
            nc.vector.tensor_copy(tr[name][:, j * 128 : (j + 1) * 128], pt)


def _group_mean_softmax(nc, pool, psum_s, tr, g, kind):
    """QK^T matmuls + softmax + channel mean for h-group g -> M [128, 32]."""
    s_ps = psum_s.tile([128, 256], F32, tag="spsum", name="spsum")
    for c in range(CH):
        chalf, clo = divmod(c, 4)
        for hh in range(4):
            h = g * 4 + hh
            col = (chalf * 4 + h // 2) * 128 + clo * 32
            prow = (h % 2) * 64
            nc.tensor.matmul(
                s_ps[hh * 32 : hh * 32 + 32, c * 32 : c * 32 + 32],
                tr["q"][prow : prow + 64, col : col + 32],
                tr["k"][prow : prow + 64, col : col + 32],
                start=True, stop=True,
                tile_position=(prow, hh * 32),
            )
    ex = pool.tile([128, 256], F32, tag=f"ex_{kind}", name=f"ex_{kind}")
    nc.scalar.activation(ex, s_ps, mybir.ActivationFunctionType.Exp, scale=1.0 / 8.0)
    r = pool.tile([128, 8], F32, tag=f"r_{kind}", name=f"r_{kind}")
    ex_cview = bass.AP(tensor=ex.tensor, offset=ex.offset,
                       ap=[list(ex.ap[0]), [32, 8], [1, 32]])
    nc.vector.tensor_reduce(r, ex_cview, axis=mybir.AxisListType.X, op=mybir.AluOpType.add)
    w = pool.tile([128, 8], F32, tag=f"w_{kind}", name=f"w_{kind}")
    nc.vector.reciprocal(w, r)
    wx = pool.tile([128, 256], F32, tag=f"wx_{kind}", name=f"wx_{kind}")
    ex_scl = bass.AP(tensor=ex.tensor, offset=ex.offset,
                     ap=[list(ex.ap[0]), [1, 32], [32, 8]])
    w_bc = bass.AP(tensor=w.tensor, offset=w.offset,
                   ap=[list(w.ap[0]), [0, 32], [1, 8]])
    wx_out = bass.AP(tensor=wx.tensor, offset=wx.offset,
                     ap=[list(wx.ap[0]), [8, 32], [1, 8]])
    nc.vector.scalar_tensor_tensor(out=wx_out, in0=ex_scl, scalar=1.0 / CH, in1=w_bc,
                                   op0=mybir.AluOpType.mult, op1=mybir.AluOpType.mult)
    m = pool.tile([128, 32], F32, tag=f"m_{kind}_{g}", name=f"m_{kind}_{g}")
    wx_in = bass.AP(tensor=wx.tensor, offset=wx.offset,
                    ap=[list(wx.ap[0]), [8, 32], [1, 8]])
    nc.vector.tensor_reduce(m, wx_in, axis=mybir.AxisListType.X, op=mybir.AluOpType.add)
    return m


def _build_ps_tile(nc, pool, m, g):
    """Element-repeat expansion tile [128, 2048] f16 (two identical 1024-col
    copies) for h-group g: doubles the DMA descriptor size to 4 KiB."""
    exp_t = pool.tile([128, 2048], F16, tag=f"expand_ps_{g}", name=f"expand_ps_{g}")
    src = bass.AP(tensor=m.tensor, offset=m.offset,
                  ap=[list(m.ap[0]), [1, 32], [0, 32]])
    for copy in range(2):
        nc.vector.tensor_copy(exp_t[:, copy * 1024 : (copy + 1) * 1024], src)
    return exp_t


def _build_pn_tiles(nc, pool, psum_r, repmats, m, g):
    """Tile expansion [128, 1024] f16 + per-head partition replication on the
    PE: rep2[p, j*1024 + c] = exp[hh*32 + (2p+j)%32, c] via two matmuls with
    the R2j permuted identities, then DVE PSUM->SBUF f16 copies. Each rep2
    tile holds 256 consecutive output rows. No SDMA involvement."""
    exp_t = pool.tile([128, 1024], F16, tag=f"expand_pn_{g}", name=f"expand_pn_{g}")
    src = bass.AP(tensor=m.tensor, offset=m.offset,
                  ap=[list(m.ap[0]), [0, 32], [1, 32]])
    nc.vector.tensor_copy(exp_t, src)
    reps = []
    for hh in range(4):
        rep = pool.tile([128, 2048], F16, tag=f"rep_pn_{g * 4 + hh}",
                        name=f"rep_pn_{g * 4 + hh}")
        for j in range(2):
            pr = psum_r.tile([128, 1024], F32, tag="rep_psum", name="rep_psum")
            for half in range(2):
                nc.tensor.matmul(
                    pr[:, half * 512 : (half + 1) * 512],
                    repmats[j][hh * 32 : (hh + 1) * 32, :],
                    exp_t[hh * 32 : (hh + 1) * 32, half * 512 : (half + 1) * 512],
                    start=True, stop=True,
                    tile_position=(hh * 32, 0),
                )
            nc.vector.tensor_copy(rep[:, j * 1024 : (j + 1) * 1024], pr)
        reps.append(rep)
    return reps


def _write_ps_group(nc, exp_t, out_dram, g, nparts=128):
    """One DMA per h-group: each 2-row source line re-read 16x (stride-0 mid
    dim) -> 32 identical consecutive output rows per partition; the HBM walk
    is fully sequential over the group's [nparts*32, 1024] row span."""
    pitch = exp_t.ap[0][0]
    src = bass.AP(tensor=exp_t.tensor, offset=exp_t.offset,
                  ap=[[pitch, nparts], [0, 16], [1, 2048]])
    dst = bass.AP(tensor=out_dram.tensor,
                  offset=out_dram.offset + g * 4 * WIN * WIN,
                  ap=[[32 * WIN, nparts], [2 * WIN, 16], [1, 2 * WIN]])
    nc.sync.dma_start(out=dst, in_=src)


def _make_dupmat(nc, pool, ident):
    """Rd [128, 128] f16 on partitions 96-127: Rd[96+s, 16a+b] = (s == 24+a).
    lhsT of the tail-duplicate matmul: dup[q] = exp_ps_g1[120 + q//16]."""
    rd = pool.tile([128, 128], F16, tag="dupmat", name="dupmat")
    ipitch = ident.ap[0][0]
    rpitch = rd.ap[0][0]
    src = bass.AP(tensor=ident.tensor, offset=ident.offset + 96 * ipitch + 120,
                  ap=[[ipitch, 32], [1, 8], [0, 16]])
    dst = bass.AP(tensor=rd.tensor, offset=rd.offset + 96 * rpitch,
                  ap=[[rpitch, 32], [16, 8], [1, 16]])
    nc.vector.tensor_copy(dst, src)
    return rd


def _build_dup_tile(nc, pool, psum_r, dupmat, exp_t):
    """dup [128, 2048] f16 with dup[q] = exp_ps_g1[120 + q//16]: the content
    of output rows 3840..4095 of h-group 1 (head 7, rows 768..1023), spread
    over all 128 partitions so its write uses all 16 SDMA engines."""
    dup = pool.tile([128, 2048], F16, tag="dup_ps", name="dup_ps")
    for r in range(2):
        pr = psum_r.tile([128, 1024], F32, tag="rep_psum", name="rep_psum")
        for half in range(2):
            nc.tensor.matmul(
                pr[:, half * 512 : (half + 1) * 512],
                dupmat[96:128, :],
                exp_t[96:128, r * 1024 + half * 512 : r * 1024 + (half + 1) * 512],
                start=True, stop=True,
                tile_position=(96, 0),
            )
        nc.vector.tensor_copy(dup[:, r * 1024 : (r + 1) * 1024], pr)
    return dup


def _write_ps_dup(nc, dup, out_dram):
    """0.5 MB tail DMA: rows 7936..8191 of out_ps (head 7 rows 768..1023)
    from the 128-partition dup tile; fully sequential dst walk (2 rows per
    partition)."""
    pitch = dup.ap[0][0]
    src = bass.AP(tensor=dup.tensor, offset=dup.offset,
                  ap=[[pitch, 128], [1, 2048]])
    dst = bass.AP(tensor=out_dram.tensor,
                  offset=out_dram.offset + (4096 + 3840) * WIN,
                  ap=[[2 * WIN, 128], [1, 2 * WIN]])
    nc.sync.dma_start(out=dst, in_=src)


def _write_pn_head(nc, rep, out_dram, h):
    """One 2 MB DMA per head: the [128, 2048] rep2 tile (256 rows) re-read 4x
    via a stride-0 mid src dim. The dst walk is 4 interleaved sequential
    streams of 4 KB runs (partition p writes rows 2p,2p+1 of each 256-row
    band), which HBM handles near-sequentially."""
    pitch = rep.ap[0][0]
    src = bass.AP(tensor=rep.tensor, offset=rep.offset,
                  ap=[[pitch, 128], [0, 4], [1, 2048]])
    dst = bass.AP(tensor=out_dram.tensor,
                  offset=out_dram.offset + h * WIN * WIN,
                  ap=[[2 * WIN, 128], [256 * WIN, 4], [1, 2 * WIN]])
    nc.sync.dma_start(out=dst, in_=src)


def build_program():
    """Build and compile the per-core Bass program. Returns the Bacc object."""
    nc = bacc.Bacc(
        "TRN2",
        target_bir_lowering=False,
        debug=False,
        enable_asserts=False,
        num_devices=N_CORES,
    )
    ins = {}
    for name in ("qps", "qpn", "kps", "kpn"):
        ins[name] = nc.dram_tensor(name, [CH, L, H, E], F32, kind="ExternalInput").ap()
    out_ps = nc.dram_tensor("out_ps", [H, WIN, WIN], F16, kind="ExternalOutput").ap()
    out_pn = nc.dram_tensor("out_pn", [H, WIN, WIN], F16, kind="ExternalOutput").ap()

    with tile.TileContext(nc) as tc:
        with ExitStack() as ctx:
            pool = ctx.enter_context(tc.tile_pool(name="sbuf", bufs=1))
            chunk_pool = ctx.enter_context(tc.tile_pool(name="chunks", bufs=2))
            psum_t = ctx.enter_context(tc.tile_pool(name="ptrans", bufs=2, space="PSUM"))
            psum_s = ctx.enter_context(tc.tile_pool(name="spsum", bufs=2, space="PSUM"))
            psum_r = ctx.enter_context(tc.tile_pool(name="rpsum", bufs=2, space="PSUM"))
            ident = pool.tile([128, 128], F32, tag="ident")
            make_identity(nc, ident)
            repmats = _make_repmat(nc, pool, ident)
            dupmat = _make_dupmat(nc, pool, ident)

            nat_ps = _load_inputs(nc, pool, {"q": ins["qps"], "k": ins["kps"]}, "ps")
            nat_pn = _load_inputs(nc, pool, {"q": ins["qpn"], "k": ins["kpn"]}, "pn")
            tr_ps = {n: pool.tile([128, 1024], F16, tag=f"tr_ps_{n}",
                                  name=f"tr_ps_{n}") for n in ("q", "k")}
            tr_pn = {n: pool.tile([128, 1024], F16, tag=f"tr_pn_{n}",
                                  name=f"tr_pn_{n}") for n in ("q", "k")}

            # Pipelined: each write is queued the moment its source tile
            # exists; pn rep tiles are built on PE+DVE while the sync ring
            # drains earlier writes. Stream order [ps g0, pn h0-3, pn h4-7,
            # ps g1] with the final ps group written from partitions [0,120)
            # (120 = 15x8 partitions -> HWDGE spreads it over engines 0-14,
            # skipping SDMA engine 15 which runs ~13% slower than the rest)
            # plus a 128-partition dup tile for the last 256 rows. All 16
            # engines then finish together instead of engine 15 dribbling
            # out a 12 us serial tail.

            # group 0: ps then pn
            _transpose_group(nc, psum_t, ident, nat_ps, tr_ps, 0)
            m = _group_mean_softmax(nc, chunk_pool, psum_s, tr_ps, 0, "ps")
            exp_ps = _build_ps_tile(nc, pool, m, 0)
            _write_ps_group(nc, exp_ps, out_ps, 0)

            _transpose_group(nc, psum_t, ident, nat_pn, tr_pn, 0)
            m = _group_mean_softmax(nc, chunk_pool, psum_s, tr_pn, 0, "pn")
            reps = _build_pn_tiles(nc, pool, psum_r, repmats, m, 0)
            for hh in range(4):
                _write_pn_head(nc, reps[hh], out_pn, hh)

            # group 1: pn first, ps (with port-15 shedding) last
            _transpose_group(nc, psum_t, ident, nat_pn, tr_pn, 1)
            m = _group_mean_softmax(nc, chunk_pool, psum_s, tr_pn, 1, "pn")
            reps = _build_pn_tiles(nc, pool, psum_r, repmats, m, 1)
            for hh in range(4):
                _write_pn_head(nc, reps[hh], out_pn, 4 + hh)

            _transpose_group(nc, psum_t, ident, nat_ps, tr_ps, 1)
            m = _group_mean_softmax(nc, chunk_pool, psum_s, tr_ps, 1, "ps")
            exp_ps = _build_ps_tile(nc, pool, m, 1)
            dup = _build_dup_tile(nc, pool, psum_r, dupmat, exp_ps)
            _write_ps_dup(nc, dup, out_ps)
            _write_ps_group(nc, exp_ps, out_ps, 1, nparts=120)
    nc.compile()
    return nc


_NC_CACHE = None


def _get_nc():
    global _NC_CACHE
    if _NC_CACHE is None:
        _NC_CACHE = build_program()
    return _NC_CACHE


def run_sharded(queries_patch_size, queries_patch_num, keys_patch_size, keys_patch_num,
                trace=False, tmpdir=None):
    """Run the SPMD kernel on 8 cores; returns (full_ps, full_pn[, results])."""
    from concourse.bass_utils import run_bass_kernel_spmd

    nc = _get_nc()
    qps = np.ascontiguousarray(np.asarray(queries_patch_size, dtype=np.float32))
    qpn = np.ascontiguousarray(np.asarray(queries_patch_num, dtype=np.float32))
    kps = np.ascontiguousarray(np.asarray(keys_patch_size, dtype=np.float32))
    kpn = np.ascontiguousarray(np.asarray(keys_patch_num, dtype=np.float32))

    in_maps = []
    for b in range(N_CORES):
        sl = slice(b * CH, (b + 1) * CH)
        in_maps.append({
            "qps": qps[sl], "qpn": qpn[sl], "kps": kps[sl], "kpn": kpn[sl],
        })
    res = run_bass_kernel_spmd(nc, in_maps, core_ids=list(range(N_CORES)), trace=trace,
                               tmpdir=tmpdir)
    full_ps = np.stack([np.asarray(res.results[b]["out_ps"]).astype(np.float32)
                        for b in range(N_CORES)], axis=0)
    full_pn = np.stack([np.asarray(res.results[b]["out_pn"]).astype(np.float32)
                        for b in range(N_CORES)], axis=0)
    if trace:
        return full_ps, full_pn, res
    return full_ps, full_pn


def kernel(queries_patch_size, queries_patch_num, keys_patch_size, keys_patch_num,
           values=None, patch_index=0, attn_mask=None):
    """Full-input entry point: takes the unsharded inputs, returns full outputs."""
    full_ps, full_pn = run_sharded(
        queries_patch_size, queries_patch_num, keys_patch_size, keys_patch_num
    )
    return full_ps, full_pn


# revision 19
# speedup vs baseline: 1.7623x; 1.0409x over previous
"""Trainium2 Bass kernel for nn_DAC_structure (sparse dual-attention structure map).

For inputs q/k of shape (B*CH, L, H, E) = (64, 32, 8, 64):
  s  = softmax((q @ k^T) / sqrt(E))            per (batch-channel, head)
  m  = mean over the CH=8 channel group        -> [b, H, 32, 32]
  out_ps = element-repeat(m_ps, 32, 32)        -> [b, H, 1024, 1024]
  out_pn = tile(m_pn, 32, 32)                  -> [b, H, 1024, 1024]

Sharding: data-parallel over the true batch dim b = 8; core i handles batch i
(channel rows 8i..8i+8). No cross-device comms. Each core writes its own
[8, 1024, 1024] x2 output shard; the host stacks shards along axis 0.

The kernel is HBM-write-bound. The HBM port per NeuronCore caps at ~358 GB/s
(716 GB/s per stack, 2 NCs/stack, all 8 cores active), so the f32 output
(64 MB/core) floors at ~187 us. This version writes fp16 instead (32 MB/core,
~94 us floor; rel err ~5e-4 vs the 2e-2 gate) and upcasts on the host during
the gather. Structure:
  - All output DMAs walk HBM fully sequentially: out_ps re-reads each source
    row 32x via a stride-0 mid AP dim; out_pn re-reads a partition-replicated
    [128, 1024] tile 8x via a stride-0 OUTER AP dim (dst merges to one
    contiguous 2 MB walk per head).
  - out_pn partition replication (32 rows -> 128) is done on the PE with a
    block-replicated identity (rep = R^T @ exp), not SBUF->SBUF DMA, so the
    SDMA engines spend all their time on HBM writes.
  - QK^T matmuls run in fp16 (tr tiles cast during the PSUM->SBUF transpose
    copy); softmax + channel-mean stay f32.
  - Everything is issued so the sync-ring write stream starts as soon as the
    first h-group's ps tile is ready (~12 us in) and never gaps.
"""

import sys

if "/opt/trn_rl_repo" not in sys.path:
    sys.path.insert(0, "/opt/trn_rl_repo")

from contextlib import ExitStack

import numpy as np

import concourse.bacc as bacc
import concourse.bass as bass
import concourse.mybir as mybir
import concourse.tile as tile
from concourse.masks import make_identity

F32 = mybir.dt.float32
F16 = mybir.dt.float16

CH = 8   # channels per true batch
L = 32   # patch_num (seq len of the small attention)
H = 8    # heads
E = 64   # head dim
WIN = 1024
N_CORES = 8


def _load_inputs(nc, pool, ins, kind):
    """Two DMAs (one per channel-half) per tensor into
    [128 = (c%4)*32 + l, 1024 = (c//4)*512 + h*64 + e]; q on the Scalar ring,
    k on the Sync ring (ahead of the output writes) so the load drains 2x
    faster and the first transpose starts earlier."""
    nat = {}
    for name, eng in (("q", nc.scalar), ("k", nc.sync)):
        dram = ins[name]
        t = pool.tile([128, 1024], F32, tag=f"nat_{kind}_{name}", name=f"nat_{kind}_{name}")
        for chalf in range(2):
            src = bass.AP(tensor=dram.tensor, offset=dram.offset + chalf * 4 * L * H * E,
                          ap=[[H * E, 128], [1, H * E]])
            eng.dma_start(out=t[:, chalf * 512 : (chalf + 1) * 512], in_=src)
        nat[name] = t
    return nat


def _make_repmat(nc, pool, ident):
    """R2j [128, 128] f16 (j=0,1) with R2j[q, p] = (q%32 == (2p+j)%32): lhsT
    of the pn partition-replication matmuls. rep2 tile row content for
    partition p, half j is exp row (2p+j)%32, so a [128, 2048] tile covers
    256 consecutive output rows."""
    ipitch = ident.ap[0][0]
    rs = []
    for j in range(2):
        r = pool.tile([128, 128], F16, tag=f"repmat{j}", name=f"repmat{j}")
        rpitch = r.ap[0][0]
        for jb in range(4):
            src = bass.AP(tensor=ident.tensor,
                          offset=ident.offset + jb * 32 * ipitch + jb * 32 + j,
                          ap=[[ipitch, 32], [0, 8], [2, 16]])
            dst = bass.AP(tensor=r.tensor, offset=r.offset + jb * 32 * rpitch,
                          ap=[[rpitch, 32], [16, 8], [1, 16]])
            nc.vector.tensor_copy(dst, src)
        rs.append(r)
    return rs


def _transpose_group(nc, psum_t, ident, nat, tr, g):
    """PE-transpose the 128-col chunks needed by h-group g (h = 4g..4g+3).
    The PSUM->SBUF copy casts f32 -> f16 so the QK^T matmuls run in fp16."""
    js = (0, 1, 4, 5) if g == 0 else (2, 3, 6, 7)
    for name in ("q", "k"):
        for j in js:
            pt = psum_t.tile([128, 128], F32, tag="ptrans", name="ptrans")
            nc.tensor.transpose(pt, nat[name][:, j * 128 : (j + 1) * 128], ident)
            nc.vector.tensor_copy(tr[name][:, j * 128 : (j + 1) * 128], pt)


def _group_mean_softmax(nc, pool, psum_s, tr, g, kind):
    """QK^T matmuls + softmax + channel mean for h-group g -> M [128, 32]."""
    s_ps = psum_s.tile([128, 256], F32, tag="spsum", name="spsum")
    for c in range(CH):
        chalf, clo = divmod(c, 4)
        for hh in range(4):
            h = g * 4 + hh
            col = (chalf * 4 + h // 2) * 128 + clo * 32
            prow = (h % 2) * 64
            nc.tensor.matmul(
                s_ps[hh * 32 : hh * 32 + 32, c * 32 : c * 32 + 32],
                tr["q"][prow : prow + 64, col : col + 32],
                tr["k"][prow : prow + 64, col : col + 32],
                start=True, stop=True,
                tile_position=(prow, hh * 32),
            )
    ex = pool.tile([128, 256], F32, tag=f"ex_{kind}", name=f"ex_{kind}")
    nc.scalar.activation(ex, s_ps, mybir.ActivationFunctionType.Exp, scale=1.0 / 8.0)
    r = pool.tile([128, 8], F32, tag=f"r_{kind}", name=f"r_{kind}")
    ex_cview = bass.AP(tensor=ex.tensor, offset=ex.offset,
                       ap=[list(ex.ap[0]), [32, 8], [1, 32]])
    nc.vector.tensor_reduce(r, ex_cview, axis=mybir.AxisListType.X, op=mybir.AluOpType.add)
    w = pool.tile([128, 8], F32, tag=f"w_{kind}", name=f"w_{kind}")
    nc.vector.reciprocal(w, r)
    wx = pool.tile([128, 256], F32, tag=f"wx_{kind}", name=f"wx_{kind}")
    ex_scl = bass.AP(tensor=ex.tensor, offset=ex.offset,
                     ap=[list(ex.ap[0]), [1, 32], [32, 8]])
    w_bc = bass.AP(tensor=w.tensor, offset=w.offset,
                   ap=[list(w.ap[0]), [0, 32], [1, 8]])
    wx_out = bass.AP(tensor=wx.tensor, offset=wx.offset,
                     ap=[list(wx.ap[0]), [8, 32], [1, 8]])
    nc.vector.scalar_tensor_tensor(out=wx_out, in0=ex_scl, scalar=1.0 / CH, in1=w_bc,
                                   op0=mybir.AluOpType.mult, op1=mybir.AluOpType.mult)
    m = pool.tile([128, 32], F32, tag=f"m_{kind}_{g}", name=f"m_{kind}_{g}")
    wx_in = bass.AP(tensor=wx.tensor, offset=wx.offset,
                    ap=[list(wx.ap[0]), [8, 32], [1, 8]])
    nc.vector.tensor_reduce(m, wx_in, axis=mybir.AxisListType.X, op=mybir.AluOpType.add)
    return m


def _build_ps_tile(nc, pool, m, g):
    """Element-repeat expansion tile [128, 2048] f16 (two identical 1024-col
    copies) for h-group g: doubles the DMA descriptor size to 4 KiB."""
    exp_t = pool.tile([128, 2048], F16, tag=f"expand_ps_{g}", name=f"expand_ps_{g}")
    src = bass.AP(tensor=m.tensor, offset=m.offset,
                  ap=[list(m.ap[0]), [1, 32], [0, 32]])
    for copy in range(2):
        nc.vector.tensor_copy(exp_t[:, copy * 1024 : (copy + 1) * 1024], src)
    return exp_t


def _build_pn_tiles(nc, pool, psum_r, repmats, m, g):
    """Tile expansion [128, 1024] f16 + per-head partition replication on the
    PE: rep2[p, j*1024 + c] = exp[hh*32 + (2p+j)%32, c] via two matmuls with
    the R2j permuted identities, then DVE PSUM->SBUF f16 copies. Each rep2
    tile holds 256 consecutive output rows. No SDMA involvement."""
    exp_t = pool.tile([128, 1024], F16, tag=f"expand_pn_{g}", name=f"expand_pn_{g}")
    src = bass.AP(tensor=m.tensor, offset=m.offset,
                  ap=[list(m.ap[0]), [0, 32], [1, 32]])
    nc.vector.tensor_copy(exp_t, src)
    reps = []
    for hh in range(4):
        rep = pool.tile([128, 2048], F16, tag=f"rep_pn_{g * 4 + hh}",
                        name=f"rep_pn_{g * 4 + hh}")
        for j in range(2):
            pr = psum_r.tile([128, 1024], F32, tag="rep_psum", name="rep_psum")
            for half in range(2):
                nc.tensor.matmul(
                    pr[:, half * 512 : (half + 1) * 512],
                    repmats[j][hh * 32 : (hh + 1) * 32, :],
                    exp_t[hh * 32 : (hh + 1) * 32, half * 512 : (half + 1) * 512],
                    start=True, stop=True,
                    tile_position=(hh * 32, 0),
                )
            nc.vector.tensor_copy(rep[:, j * 1024 : (j + 1) * 1024], pr)
        reps.append(rep)
    return reps


def _write_ps_group(nc, exp_t, out_dram, g, nparts=128):
    """One DMA per h-group: each 2-row source line re-read 16x (stride-0 mid
    dim) -> 32 identical consecutive output rows per partition; the HBM walk
    is fully sequential over the group's [nparts*32, 1024] row span."""
    pitch = exp_t.ap[0][0]
    src = bass.AP(tensor=exp_t.tensor, offset=exp_t.offset,
                  ap=[[pitch, nparts], [0, 16], [1, 2048]])
    dst = bass.AP(tensor=out_dram.tensor,
                  offset=out_dram.offset + g * 4 * WIN * WIN,
                  ap=[[32 * WIN, nparts], [2 * WIN, 16], [1, 2 * WIN]])
    nc.sync.dma_start(out=dst, in_=src)


def _write_pn_head(nc, rep, out_dram, h):
    """One 2 MB DMA per head: the [128, 2048] rep2 tile (256 rows) re-read 4x
    via a stride-0 mid src dim. The dst walk is 4 interleaved sequential
    streams of 4 KB runs (partition p writes rows 2p,2p+1 of each 256-row
    band), which HBM handles near-sequentially."""
    pitch = rep.ap[0][0]
    src = bass.AP(tensor=rep.tensor, offset=rep.offset,
                  ap=[[pitch, 128], [0, 4], [1, 2048]])
    dst = bass.AP(tensor=out_dram.tensor,
                  offset=out_dram.offset + h * WIN * WIN,
                  ap=[[2 * WIN, 128], [256 * WIN, 4], [1, 2 * WIN]])
    nc.sync.dma_start(out=dst, in_=src)


def build_program():
    """Build and compile the per-core Bass program. Returns the Bacc object."""
    nc = bacc.Bacc(
        "TRN2",
        target_bir_lowering=False,
        debug=False,
        enable_asserts=False,
        num_devices=N_CORES,
    )
    ins = {}
    for name in ("qps", "qpn", "kps", "kpn"):
        ins[name] = nc.dram_tensor(name, [CH, L, H, E], F32, kind="ExternalInput").ap()
    out_ps = nc.dram_tensor("out_ps", [H, WIN, WIN], F16, kind="ExternalOutput").ap()
    out_pn = nc.dram_tensor("out_pn", [H, WIN, WIN], F16, kind="ExternalOutput").ap()

    with tile.TileContext(nc) as tc:
        with ExitStack() as ctx:
            pool = ctx.enter_context(tc.tile_pool(name="sbuf", bufs=1))
            chunk_pool = ctx.enter_context(tc.tile_pool(name="chunks", bufs=2))
            psum_t = ctx.enter_context(tc.tile_pool(name="ptrans", bufs=2, space="PSUM"))
            psum_s = ctx.enter_context(tc.tile_pool(name="spsum", bufs=2, space="PSUM"))
            psum_r = ctx.enter_context(tc.tile_pool(name="rpsum", bufs=2, space="PSUM"))
            ident = pool.tile([128, 128], F32, tag="ident")
            make_identity(nc, ident)
            repmats = _make_repmat(nc, pool, ident)

            nat_ps = _load_inputs(nc, pool, {"q": ins["qps"], "k": ins["kps"]}, "ps")
            nat_pn = _load_inputs(nc, pool, {"q": ins["qpn"], "k": ins["kpn"]}, "pn")
            tr_ps = {n: pool.tile([128, 1024], F16, tag=f"tr_ps_{n}",
                                  name=f"tr_ps_{n}") for n in ("q", "k")}
            tr_pn = {n: pool.tile([128, 1024], F16, tag=f"tr_pn_{n}",
                                  name=f"tr_pn_{n}") for n in ("q", "k")}

            # Pipelined: each write is queued the moment its source tile
            # exists; pn rep tiles are built on PE+DVE while the sync ring
            # drains earlier writes. All output DMAs use exactly 128 source
            # partitions: HWDGE runs non-128-partition transfers at roughly
            # half the per-engine packet pace (measured: a 124-partition DMA
            # collapsed to 4 engines, a 120-partition one to ~2x slower
            # packets), so partition-subset tricks lose more than they save.

            # group 0: ps then pn
            _transpose_group(nc, psum_t, ident, nat_ps, tr_ps, 0)
            m = _group_mean_softmax(nc, chunk_pool, psum_s, tr_ps, 0, "ps")
            exp_ps = _build_ps_tile(nc, pool, m, 0)
            _write_ps_group(nc, exp_ps, out_ps, 0)

            _transpose_group(nc, psum_t, ident, nat_pn, tr_pn, 0)
            m = _group_mean_softmax(nc, chunk_pool, psum_s, tr_pn, 0, "pn")
            reps = _build_pn_tiles(nc, pool, psum_r, repmats, m, 0)
            for hh in range(4):
                _write_pn_head(nc, reps[hh], out_pn, hh)

            # group 1: pn first, ps (with port-15 shedding) last
            _transpose_group(nc, psum_t, ident, nat_pn, tr_pn, 1)
            m = _group_mean_softmax(nc, chunk_pool, psum_s, tr_pn, 1, "pn")
            reps = _build_pn_tiles(nc, pool, psum_r, repmats, m, 1)
            for hh in range(4):
                _write_pn_head(nc, reps[hh], out_pn, 4 + hh)

            _transpose_group(nc, psum_t, ident, nat_ps, tr_ps, 1)
            m = _group_mean_softmax(nc, chunk_pool, psum_s, tr_ps, 1, "ps")
            exp_ps = _build_ps_tile(nc, pool, m, 1)
            _write_ps_group(nc, exp_ps, out_ps, 1)
    nc.compile()
    return nc


_NC_CACHE = None


def _get_nc():
    global _NC_CACHE
    if _NC_CACHE is None:
        _NC_CACHE = build_program()
    return _NC_CACHE


def run_sharded(queries_patch_size, queries_patch_num, keys_patch_size, keys_patch_num,
                trace=False, tmpdir=None):
    """Run the SPMD kernel on 8 cores; returns (full_ps, full_pn[, results])."""
    from concourse.bass_utils import run_bass_kernel_spmd

    nc = _get_nc()
    qps = np.ascontiguousarray(np.asarray(queries_patch_size, dtype=np.float32))
    qpn = np.ascontiguousarray(np.asarray(queries_patch_num, dtype=np.float32))
    kps = np.ascontiguousarray(np.asarray(keys_patch_size, dtype=np.float32))
    kpn = np.ascontiguousarray(np.asarray(keys_patch_num, dtype=np.float32))

    in_maps = []
    for b in range(N_CORES):
        sl = slice(b * CH, (b + 1) * CH)
        in_maps.append({
            "qps": qps[sl], "qpn": qpn[sl], "kps": kps[sl], "kpn": kpn[sl],
        })
    res = run_bass_kernel_spmd(nc, in_maps, core_ids=list(range(N_CORES)), trace=trace,
                               tmpdir=tmpdir)
    full_ps = np.stack([np.asarray(res.results[b]["out_ps"]).astype(np.float32)
                        for b in range(N_CORES)], axis=0)
    full_pn = np.stack([np.asarray(res.results[b]["out_pn"]).astype(np.float32)
                        for b in range(N_CORES)], axis=0)
    if trace:
        return full_ps, full_pn, res
    return full_ps, full_pn


def kernel(queries_patch_size, queries_patch_num, keys_patch_size, keys_patch_num,
           values=None, patch_index=0, attn_mask=None):
    """Full-input entry point: takes the unsharded inputs, returns full outputs."""
    full_ps, full_pn = run_sharded(
        queries_patch_size, queries_patch_num, keys_patch_size, keys_patch_num
    )
    return full_ps, full_pn


# revision 22
# speedup vs baseline: 1.8111x; 1.0277x over previous
"""Trainium2 Bass kernel for nn_DAC_structure (sparse dual-attention structure map).

For inputs q/k of shape (B*CH, L, H, E) = (64, 32, 8, 64):
  s  = softmax((q @ k^T) / sqrt(E))            per (batch-channel, head)
  m  = mean over the CH=8 channel group        -> [b, H, 32, 32]
  out_ps = element-repeat(m_ps, 32, 32)        -> [b, H, 1024, 1024]
  out_pn = tile(m_pn, 32, 32)                  -> [b, H, 1024, 1024]

Sharding: data-parallel over the true batch dim b = 8; core i handles batch i
(channel rows 8i..8i+8). No cross-device comms. Each core writes its own
[8, 1024, 1024] x2 output shard; the host stacks shards along axis 0.

The kernel is HBM-write-bound. The HBM port per NeuronCore caps at ~358 GB/s
(716 GB/s per stack, 2 NCs/stack, all 8 cores active), so the f32 output
(64 MB/core) floors at ~187 us. This version writes fp16 instead (32 MB/core,
~94 us floor; rel err ~5e-4 vs the 2e-2 gate) and upcasts on the host during
the gather. Structure:
  - All output DMAs walk HBM fully sequentially: out_ps re-reads each source
    row 32x via a stride-0 mid AP dim; out_pn re-reads a partition-replicated
    [128, 1024] tile 8x via a stride-0 OUTER AP dim (dst merges to one
    contiguous 2 MB walk per head).
  - out_pn partition replication (32 rows -> 128) is done on the PE with a
    block-replicated identity (rep = R^T @ exp), not SBUF->SBUF DMA, so the
    SDMA engines spend all their time on HBM writes.
  - QK^T matmuls run in fp16 (tr tiles cast during the PSUM->SBUF transpose
    copy); softmax + channel-mean stay f32.
  - Everything is issued so the sync-ring write stream starts as soon as the
    first h-group's ps tile is ready (~12 us in) and never gaps.
"""

import sys

if "/opt/trn_rl_repo" not in sys.path:
    sys.path.insert(0, "/opt/trn_rl_repo")

from contextlib import ExitStack

import numpy as np

import concourse.bacc as bacc
import concourse.bass as bass
import concourse.mybir as mybir
import concourse.tile as tile
from concourse.masks import make_identity

F32 = mybir.dt.float32
F16 = mybir.dt.float16

CH = 8   # channels per true batch
L = 32   # patch_num (seq len of the small attention)
H = 8    # heads
E = 64   # head dim
WIN = 1024
N_CORES = 8


def _load_inputs(nc, pool, ins, kind, engines):
    """Two DMAs (one per channel-half) per tensor into
    [128 = (c%4)*32 + l, 1024 = (c//4)*512 + h*64 + e]. The ps pair is split
    across the Scalar and Sync rings so the critical first load drains 2x
    faster; the pn pair stays off the Sync ring so it never sits ahead of the
    first output write."""
    nat = {}
    for name, eng in zip(("q", "k"), engines):
        dram = ins[name]
        t = pool.tile([128, 1024], F32, tag=f"nat_{kind}_{name}", name=f"nat_{kind}_{name}")
        for chalf in range(2):
            src = bass.AP(tensor=dram.tensor, offset=dram.offset + chalf * 4 * L * H * E,
                          ap=[[H * E, 128], [1, H * E]])
            eng.dma_start(out=t[:, chalf * 512 : (chalf + 1) * 512], in_=src)
        nat[name] = t
    return nat


def _make_repmat(nc, pool, ident):
    """R2j [128, 128] f16 (j=0,1) with R2j[q, p] = (q%32 == (2p+j)%32): lhsT
    of the pn partition-replication matmuls. rep2 tile row content for
    partition p, half j is exp row (2p+j)%32, so a [128, 2048] tile covers
    256 consecutive output rows."""
    ipitch = ident.ap[0][0]
    rs = []
    for j in range(2):
        r = pool.tile([128, 128], F16, tag=f"repmat{j}", name=f"repmat{j}")
        rpitch = r.ap[0][0]
        for jb in range(4):
            src = bass.AP(tensor=ident.tensor,
                          offset=ident.offset + jb * 32 * ipitch + jb * 32 + j,
                          ap=[[ipitch, 32], [0, 8], [2, 16]])
            dst = bass.AP(tensor=r.tensor, offset=r.offset + jb * 32 * rpitch,
                          ap=[[rpitch, 32], [16, 8], [1, 16]])
            nc.vector.tensor_copy(dst, src)
        rs.append(r)
    return rs


def _transpose_group(nc, psum_t, ident, nat, tr, g):
    """PE-transpose the 128-col chunks needed by h-group g (h = 4g..4g+3).
    The PSUM->SBUF copy casts f32 -> f16 so the QK^T matmuls run in fp16."""
    js = (0, 1, 4, 5) if g == 0 else (2, 3, 6, 7)
    for name in ("q", "k"):
        for j in js:
            pt = psum_t.tile([128, 128], F32, tag="ptrans", name="ptrans")
            nc.tensor.transpose(pt, nat[name][:, j * 128 : (j + 1) * 128], ident)
            nc.vector.tensor_copy(tr[name][:, j * 128 : (j + 1) * 128], pt)


def _group_mean_softmax(nc, pool, psum_s, tr, g, kind):
    """QK^T matmuls + softmax + channel mean for h-group g -> M [128, 32]."""
    s_ps = psum_s.tile([128, 256], F32, tag="spsum", name="spsum")
    for c in range(CH):
        chalf, clo = divmod(c, 4)
        for hh in range(4):
            h = g * 4 + hh
            col = (chalf * 4 + h // 2) * 128 + clo * 32
            prow = (h % 2) * 64
            nc.tensor.matmul(
                s_ps[hh * 32 : hh * 32 + 32, c * 32 : c * 32 + 32],
                tr["q"][prow : prow + 64, col : col + 32],
                tr["k"][prow : prow + 64, col : col + 32],
                start=True, stop=True,
                tile_position=(prow, hh * 32),
            )
    ex = pool.tile([128, 256], F32, tag=f"ex_{kind}", name=f"ex_{kind}")
    nc.scalar.activation(ex, s_ps, mybir.ActivationFunctionType.Exp, scale=1.0 / 8.0)
    r = pool.tile([128, 8], F32, tag=f"r_{kind}", name=f"r_{kind}")
    ex_cview = bass.AP(tensor=ex.tensor, offset=ex.offset,
                       ap=[list(ex.ap[0]), [32, 8], [1, 32]])
    nc.vector.tensor_reduce(r, ex_cview, axis=mybir.AxisListType.X, op=mybir.AluOpType.add)
    w = pool.tile([128, 8], F32, tag=f"w_{kind}", name=f"w_{kind}")
    nc.vector.reciprocal(w, r)
    wx = pool.tile([128, 256], F32, tag=f"wx_{kind}", name=f"wx_{kind}")
    ex_scl = bass.AP(tensor=ex.tensor, offset=ex.offset,
                     ap=[list(ex.ap[0]), [1, 32], [32, 8]])
    w_bc = bass.AP(tensor=w.tensor, offset=w.offset,
                   ap=[list(w.ap[0]), [0, 32], [1, 8]])
    wx_out = bass.AP(tensor=wx.tensor, offset=wx.offset,
                     ap=[list(wx.ap[0]), [8, 32], [1, 8]])
    nc.vector.scalar_tensor_tensor(out=wx_out, in0=ex_scl, scalar=1.0 / CH, in1=w_bc,
                                   op0=mybir.AluOpType.mult, op1=mybir.AluOpType.mult)
    m = pool.tile([128, 32], F32, tag=f"m_{kind}_{g}", name=f"m_{kind}_{g}")
    wx_in = bass.AP(tensor=wx.tensor, offset=wx.offset,
                    ap=[list(wx.ap[0]), [8, 32], [1, 8]])
    nc.vector.tensor_reduce(m, wx_in, axis=mybir.AxisListType.X, op=mybir.AluOpType.add)
    return m


def _build_ps_tile(nc, pool, m, g):
    """Element-repeat expansion tile [128, 2048] f16 (two identical 1024-col
    copies) for h-group g: doubles the DMA descriptor size to 4 KiB. Copy 2
    duplicates copy 1 at the fast f16->f16 DVE rate to keep the serial path
    to the first output DMA short."""
    exp_t = pool.tile([128, 2048], F16, tag=f"expand_ps_{g}", name=f"expand_ps_{g}")
    src = bass.AP(tensor=m.tensor, offset=m.offset,
                  ap=[list(m.ap[0]), [1, 32], [0, 32]])
    nc.vector.tensor_copy(exp_t[:, 0:1024], src)
    nc.vector.tensor_copy(exp_t[:, 1024:2048], exp_t[:, 0:1024])
    return exp_t


def _build_pn_tiles(nc, pool, psum_r, repmats, m, g):
    """Tile expansion [128, 1024] f16 + per-head partition replication on the
    PE: rep2[p, j*1024 + c] = exp[hh*32 + (2p+j)%32, c] via two matmuls with
    the R2j permuted identities, then DVE PSUM->SBUF f16 copies. Each rep2
    tile holds 256 consecutive output rows. No SDMA involvement."""
    exp_t = pool.tile([128, 1024], F16, tag=f"expand_pn_{g}", name=f"expand_pn_{g}")
    src = bass.AP(tensor=m.tensor, offset=m.offset,
                  ap=[list(m.ap[0]), [0, 32], [1, 32]])
    nc.vector.tensor_copy(exp_t, src)
    reps = []
    for hh in range(4):
        rep = pool.tile([128, 2048], F16, tag=f"rep_pn_{g * 4 + hh}",
                        name=f"rep_pn_{g * 4 + hh}")
        for j in range(2):
            pr = psum_r.tile([128, 1024], F32, tag="rep_psum", name="rep_psum")
            for half in range(2):
                nc.tensor.matmul(
                    pr[:, half * 512 : (half + 1) * 512],
                    repmats[j][hh * 32 : (hh + 1) * 32, :],
                    exp_t[hh * 32 : (hh + 1) * 32, half * 512 : (half + 1) * 512],
                    start=True, stop=True,
                    tile_position=(hh * 32, 0),
                )
            nc.vector.tensor_copy(rep[:, j * 1024 : (j + 1) * 1024], pr)
        reps.append(rep)
    return reps


def _write_ps_group(nc, exp_t, out_dram, g, nparts=128):
    """One DMA per h-group: each 2-row source line re-read 16x (stride-0 mid
    dim) -> 32 identical consecutive output rows per partition; the HBM walk
    is fully sequential over the group's [nparts*32, 1024] row span."""
    pitch = exp_t.ap[0][0]
    src = bass.AP(tensor=exp_t.tensor, offset=exp_t.offset,
                  ap=[[pitch, nparts], [0, 16], [1, 2048]])
    dst = bass.AP(tensor=out_dram.tensor,
                  offset=out_dram.offset + g * 4 * WIN * WIN,
                  ap=[[32 * WIN, nparts], [2 * WIN, 16], [1, 2 * WIN]])
    nc.sync.dma_start(out=dst, in_=src)


def _write_pn_head(nc, rep, out_dram, h):
    """One 2 MB DMA per head: the [128, 2048] rep2 tile (256 rows) re-read 4x
    via a stride-0 mid src dim. The dst walk is 4 interleaved sequential
    streams of 4 KB runs (partition p writes rows 2p,2p+1 of each 256-row
    band), which HBM handles near-sequentially."""
    pitch = rep.ap[0][0]
    src = bass.AP(tensor=rep.tensor, offset=rep.offset,
                  ap=[[pitch, 128], [0, 4], [1, 2048]])
    dst = bass.AP(tensor=out_dram.tensor,
                  offset=out_dram.offset + h * WIN * WIN,
                  ap=[[2 * WIN, 128], [256 * WIN, 4], [1, 2 * WIN]])
    nc.sync.dma_start(out=dst, in_=src)


def build_program():
    """Build and compile the per-core Bass program. Returns the Bacc object."""
    nc = bacc.Bacc(
        "TRN2",
        target_bir_lowering=False,
        debug=False,
        enable_asserts=False,
        num_devices=N_CORES,
    )
    ins = {}
    for name in ("qps", "qpn", "kps", "kpn"):
        ins[name] = nc.dram_tensor(name, [CH, L, H, E], F32, kind="ExternalInput").ap()
    out_ps = nc.dram_tensor("out_ps", [H, WIN, WIN], F16, kind="ExternalOutput").ap()
    out_pn = nc.dram_tensor("out_pn", [H, WIN, WIN], F16, kind="ExternalOutput").ap()

    with tile.TileContext(nc) as tc:
        with ExitStack() as ctx:
            pool = ctx.enter_context(tc.tile_pool(name="sbuf", bufs=1))
            chunk_pool = ctx.enter_context(tc.tile_pool(name="chunks", bufs=2))
            psum_t = ctx.enter_context(tc.tile_pool(name="ptrans", bufs=2, space="PSUM"))
            psum_s = ctx.enter_context(tc.tile_pool(name="spsum", bufs=2, space="PSUM"))
            psum_r = ctx.enter_context(tc.tile_pool(name="rpsum", bufs=2, space="PSUM"))
            ident = pool.tile([128, 128], F32, tag="ident")
            make_identity(nc, ident)
            repmats = _make_repmat(nc, pool, ident)

            nat_ps = _load_inputs(nc, pool, {"q": ins["qps"], "k": ins["kps"]}, "ps",
                                  (nc.scalar, nc.sync))
            nat_pn = _load_inputs(nc, pool, {"q": ins["qpn"], "k": ins["kpn"]}, "pn",
                                  (nc.scalar, nc.scalar))
            tr_ps = {n: pool.tile([128, 1024], F16, tag=f"tr_ps_{n}",
                                  name=f"tr_ps_{n}") for n in ("q", "k")}
            tr_pn = {n: pool.tile([128, 1024], F16, tag=f"tr_pn_{n}",
                                  name=f"tr_pn_{n}") for n in ("q", "k")}

            # Pipelined: each write is queued the moment its source tile
            # exists; pn rep tiles are built on PE+DVE while the sync ring
            # drains earlier writes. All output DMAs use exactly 128 source
            # partitions: HWDGE runs non-128-partition transfers at roughly
            # half the per-engine packet pace (measured: a 124-partition DMA
            # collapsed to 4 engines, a 120-partition one to ~2x slower
            # packets), so partition-subset tricks lose more than they save.

            # group 0: ps then pn
            _transpose_group(nc, psum_t, ident, nat_ps, tr_ps, 0)
            m = _group_mean_softmax(nc, chunk_pool, psum_s, tr_ps, 0, "ps")
            exp_ps = _build_ps_tile(nc, pool, m, 0)
            _write_ps_group(nc, exp_ps, out_ps, 0)

            _transpose_group(nc, psum_t, ident, nat_pn, tr_pn, 0)
            m = _group_mean_softmax(nc, chunk_pool, psum_s, tr_pn, 0, "pn")
            reps = _build_pn_tiles(nc, pool, psum_r, repmats, m, 0)
            for hh in range(4):
                _write_pn_head(nc, reps[hh], out_pn, hh)

            # group 1: pn first, ps (with port-15 shedding) last
            _transpose_group(nc, psum_t, ident, nat_pn, tr_pn, 1)
            m = _group_mean_softmax(nc, chunk_pool, psum_s, tr_pn, 1, "pn")
            reps = _build_pn_tiles(nc, pool, psum_r, repmats, m, 1)
            for hh in range(4):
                _write_pn_head(nc, reps[hh], out_pn, 4 + hh)

            _transpose_group(nc, psum_t, ident, nat_ps, tr_ps, 1)
            m = _group_mean_softmax(nc, chunk_pool, psum_s, tr_ps, 1, "ps")
            exp_ps = _build_ps_tile(nc, pool, m, 1)
            _write_ps_group(nc, exp_ps, out_ps, 1)
    nc.compile()
    return nc


_NC_CACHE = None


def _get_nc():
    global _NC_CACHE
    if _NC_CACHE is None:
        _NC_CACHE = build_program()
    return _NC_CACHE


def run_sharded(queries_patch_size, queries_patch_num, keys_patch_size, keys_patch_num,
                trace=False, tmpdir=None):
    """Run the SPMD kernel on 8 cores; returns (full_ps, full_pn[, results])."""
    from concourse.bass_utils import run_bass_kernel_spmd

    nc = _get_nc()
    qps = np.ascontiguousarray(np.asarray(queries_patch_size, dtype=np.float32))
    qpn = np.ascontiguousarray(np.asarray(queries_patch_num, dtype=np.float32))
    kps = np.ascontiguousarray(np.asarray(keys_patch_size, dtype=np.float32))
    kpn = np.ascontiguousarray(np.asarray(keys_patch_num, dtype=np.float32))

    in_maps = []
    for b in range(N_CORES):
        sl = slice(b * CH, (b + 1) * CH)
        in_maps.append({
            "qps": qps[sl], "qpn": qpn[sl], "kps": kps[sl], "kpn": kpn[sl],
        })
    res = run_bass_kernel_spmd(nc, in_maps, core_ids=list(range(N_CORES)), trace=trace,
                               tmpdir=tmpdir)
    full_ps = np.stack([np.asarray(res.results[b]["out_ps"]).astype(np.float32)
                        for b in range(N_CORES)], axis=0)
    full_pn = np.stack([np.asarray(res.results[b]["out_pn"]).astype(np.float32)
                        for b in range(N_CORES)], axis=0)
    if trace:
        return full_ps, full_pn, res
    return full_ps, full_pn


def kernel(queries_patch_size, queries_patch_num, keys_patch_size, keys_patch_num,
           values=None, patch_index=0, attn_mask=None):
    """Full-input entry point: takes the unsharded inputs, returns full outputs."""
    full_ps, full_pn = run_sharded(
        queries_patch_size, queries_patch_num, keys_patch_size, keys_patch_num
    )
    return full_ps, full_pn
